# revision 1
# baseline (speedup 1.0000x reference)
"""Trainium2 Bass kernel for CanonCausalMultiheadAttn.

Sharding: tensor-parallel over heads across 8 cores (2 q-heads + 1 kv-head
per core), both batches replicated. Each core computes its heads' attention
for both batches, then a single 8-core AllToAll exchanges attention outputs
so each core owns one (batch, seq-slice) of the final output projection.

Per-core pipeline (all shapes hardcoded for B=2, S=2048, D=2048):
  QKV proj (bf16 matmul) -> canon conv chunk-wise (DVE) -> qk rmsnorm via PE
  column-sum matmuls; q's rstd broadcast via K=1 outer-product matmul, k's
  rstd applied later as the per-partition `scale` of the exp activation ->
  RoPE (DVE, norm-weight & 1/sqrt(dh) folded into host cos/sin tables) ->
  causal attention with scores in [Sk, Sq] layout (bf16 matmul; no
  max-subtraction needed since |logit| <= sqrt(128) after qk-norm) ->
  exp (ACT, bf16 out) -> P@V with a ones-column appended to V giving row
  sums for free -> AllToAll -> output projection (bf16 matmul).
"""
import sys

sys.path.insert(0, '/opt/trn_rl_repo')

import numpy as np
import ml_dtypes

import concourse.bass as bass
import concourse.mybir as mybir
import concourse.tile as tile
from concourse import bacc
from concourse.bass_utils import run_bass_kernel_spmd

F32 = mybir.dt.float32
F32R = mybir.dt.float32r
BF16 = mybir.dt.bfloat16
AF = mybir.ActivationFunctionType
ALU = mybir.AluOpType

B, S, D = 2, 2048, 2048
NH, NKV, DH = 16, 8, 128
K_CONV = 4
EPS = 1e-6
SCALE = 1.0 / float(np.sqrt(DH))
NEG = -1e9
N_CORES = 8
N_CHUNKS = S // 512     # 512-wide chunks per batch
DEBUG = False
N_SKB = S // 128        # Sk blocks per batch
VSTR = 144              # V_aug stride per Sk block; 144*2B = 288B keeps each
                        # block 32B-aligned for the xbar DMA transpose


def _build():
    nc = bacc.Bacc("TRN2", target_bir_lowering=False, debug=False,
                   num_devices=N_CORES)

    hsT = nc.dram_tensor("hsT", [D, B * S], BF16, kind="ExternalInput")
    wT = nc.dram_tensor("wT", [D, 512], BF16, kind="ExternalInput")
    woT = nc.dram_tensor("woT", [D, D], BF16, kind="ExternalInput")
    cw = nc.dram_tensor("cw", [512, K_CONV], F32, kind="ExternalInput")
    ropeAq = nc.dram_tensor("ropeAq", [DH, S], F32, kind="ExternalInput")
    ropeBq = nc.dram_tensor("ropeBq", [DH, S], F32, kind="ExternalInput")
    ropeAk = nc.dram_tensor("ropeAk", [DH, S], F32, kind="ExternalInput")
    ropeBk = nc.dram_tensor("ropeBk", [DH, S], F32, kind="ExternalInput")
    maskd = nc.dram_tensor("maskd", [128, 128], F32, kind="ExternalInput")
    out = nc.dram_tensor("out", [512, D], F32, kind="ExternalOutput")
    dbg = {}
    if DEBUG:
        dbg["cn0"] = nc.dram_tensor("d_cn0", [128, S], F32, kind="ExternalOutput")
        dbg["cn3"] = nc.dram_tensor("d_cn3", [128, S], BF16, kind="ExternalOutput")
        dbg["roped0"] = nc.dram_tensor("d_roped0", [128, S], BF16, kind="ExternalOutput")
        dbg["roped2"] = nc.dram_tensor("d_roped2", [128, S], BF16, kind="ExternalOutput")
        dbg["rstdkT"] = nc.dram_tensor("d_rstdkT", [128, N_SKB], F32, kind="ExternalOutput")
        dbg["vaug"] = nc.dram_tensor("d_vaug", [128, N_SKB * VSTR], BF16, kind="ExternalOutput")
        dbg["bc"] = nc.dram_tensor("d_bc", [128, S], F32, kind="ExternalOutput")
        dbg["p00"] = nc.dram_tensor("d_p00", [128, 512], BF16, kind="ExternalOutput")
        dbg["ab0"] = nc.dram_tensor("d_ab0", [128, 128], BF16, kind="ExternalOutput")
        dbg["a2ain"] = nc.dram_tensor("d_a2ain", [2048, 512], BF16, kind="ExternalOutput")
        dbg["a2aout"] = nc.dram_tensor("d_a2aout", [2048, 512], BF16, kind="ExternalOutput")

    with tile.TileContext(nc) as tc:
        with tc.tile_pool(name="const", bufs=1) as cpool, \
             tc.tile_pool(name="persist", bufs=1) as pers, \
             tc.tile_pool(name="dram", bufs=1, space="DRAM") as dram:

            # ---- constants ----
            ropes = {}
            for nm, t in (("Aq", ropeAq), ("Bq", ropeBq),
                          ("Ak", ropeAk), ("Bk", ropeBk)):
                rt = cpool.tile([DH, S], F32, tag=f"rope{nm}", name=f"rope{nm}")
                nc.sync.dma_start(rt[:], t.ap())
                ropes[nm] = rt
            mask_sb = cpool.tile([128, 128], F32, tag="mask")
            nc.sync.dma_start(mask_sb[:], maskd.ap())
            cw_sb = []
            for mt in range(4):
                t = cpool.tile([128, K_CONV], F32, tag=f"cw{mt}", name=f"cw{mt}")
                nc.sync.dma_start(t[:], cw.ap()[128 * mt:128 * mt + 128, :])
                cw_sb.append(t)
            ones_col_f = cpool.tile([128, 1], F32, tag="ocf")
            nc.vector.memset(ones_col_f[:], 1.0)
            ones_col = cpool.tile([128, 1], F32R, tag="oc")
            nc.scalar.copy(ones_col[:], ones_col_f[:])
            eps_sb = cpool.tile([1, 1], F32, tag="eps")
            nc.vector.memset(eps_sb[:], EPS)
            ones_row_f = cpool.tile([1, 128], F32, tag="orf")
            nc.vector.memset(ones_row_f[:], 1.0)
            ones_row = cpool.tile([1, 128], F32R, tag="or")
            nc.scalar.copy(ones_row[:], ones_row_f[:])
            s0_sb = []
            for mt in range(4):
                t = cpool.tile([128, 1], F32, tag=f"s0{mt}", name=f"s0{mt}")
                nc.vector.tensor_scalar_add(t[:], cw_sb[mt][:, 0:1], 1.0)
                s0_sb.append(t)

            # persistent per-(b,mt) tiles
            roped = {}   # (b, mt<3) -> [128, S] bf16
            vaug = {}    # b -> [128, N_SKB*VSTR] bf16
            rstdkT = {}  # b -> [128, N_SKB] f32 (k rstd, transposed per block)

            # ============ QKV + canon + norm + rope, per batch ============
            for b in range(B):
                with tc.tile_pool(name=f"bwork{b}", bufs=1) as bw:
                    cn = {}
                    for mt in range(3):
                        cn[mt] = bw.tile([128, S], F32, tag=f"cn{mt}",
                                         name=f"cn{mt}")
                    cn[3] = bw.tile([128, S], BF16, tag="cn3", name="cn3")
                    with tc.tile_pool(name=f"qps{b}", bufs=2,
                                      space="PSUM") as qps:
                        prev_raw = {}
                        for n in range(N_CHUNKS):
                            hs_sb = bw.tile([128, 16 * 512], BF16,
                                            tag="hschunk", bufs=2,
                                            name="hs_sb")
                            nc.sync.dma_start(
                                hs_sb[:].rearrange("p (k s) -> p k s", s=512),
                                hsT.ap()[:, b * S + 512 * n:
                                         b * S + 512 * (n + 1)]
                                .rearrange("(k p) s -> p k s", p=128))
                            hv = hs_sb[:].rearrange("p (k s) -> p k s", s=512)
                            psums = [qps.tile([128, 512], F32, tag=f"qk{mt}",
                                              name=f"qk{mt}")
                                     for mt in range(4)]
                            for k in range(16):
                                wt_k = bw.tile([128, 512], BF16, tag="wtk",
                                               bufs=6, name="wt_k")
                                nc.sync.dma_start(
                                    wt_k[:],
                                    wT.ap()[128 * k:128 * (k + 1), :])
                                for mt in range(4):
                                    nc.tensor.matmul(
                                        psums[mt][:],
                                        wt_k[:, 128 * mt:128 * (mt + 1)],
                                        hv[:, k, :],
                                        start=(k == 0), stop=(k == 15))
                            # canon conv, chunk-wise from a raw copy
                            for mt in range(4):
                                raw_c = bw.tile([128, 512], F32,
                                                tag=f"rawc{mt}", bufs=2,
                                                name=f"rawc{mt}")
                                nc.scalar.copy(raw_c[:], psums[mt][:])
                                c = cn[mt]
                                lo = 512 * n
                                nc.vector.tensor_scalar_mul(
                                    c[:, lo:lo + 512], raw_c[:], s0_sb[mt][:])
                                for k in range(1, K_CONV):
                                    nc.vector.scalar_tensor_tensor(
                                        c[:, lo + k:lo + 512],
                                        raw_c[:, 0:512 - k],
                                        cw_sb[mt][:, k:k + 1],
                                        c[:, lo + k:lo + 512],
                                        ALU.mult, ALU.add)
                                    if n > 0:
                                        nc.vector.scalar_tensor_tensor(
                                            c[:, lo:lo + k],
                                            prev_raw[mt][:, 512 - k:512],
                                            cw_sb[mt][:, k:k + 1],
                                            c[:, lo:lo + k],
                                            ALU.mult, ALU.add)
                                prev_raw[mt] = raw_c

                    # V: transpose canon output into V_aug blocks
                    va = pers.tile([128, N_SKB * VSTR], BF16, tag=f"vaug{b}",
                                   name=f"vaug{b}")
                    vaug[b] = va
                    for i in range(N_SKB):
                        nc.sync.dma_start_transpose(
                            va[:, VSTR * i:VSTR * i + 128],
                            cn[3][:, 128 * i:128 * (i + 1)])
                    nc.vector.memset(
                        va[:].rearrange("p (i c) -> p i c",
                                        c=VSTR)[:, :, 128:129], 1.0)

                    # rmsnorm rstd + rope for q0, q1, k
                    rkt = pers.tile([128, N_SKB], F32, tag=f"rstdkT{b}",
                                    name=f"rstdkT{b}")
                    rstdkT[b] = rkt
                    rk_d = dram.tile([N_SKB, 128], F32, tag=f"rkd{b}",
                                     name=f"rk_d{b}")
                    with tc.tile_pool(name=f"nps{b}", bufs=2,
                                      space="PSUM") as nps, \
                         tc.tile_pool(name=f"bps{b}", bufs=2,
                                      space="PSUM") as bps:
                        for mt in range(3):
                            x = cn[mt]
                            is_q = mt < 2
                            bc = None
                            if is_q:
                                bc = bw.tile([128, S], F32, tag="bc",
                                             name="bc")
                            for c in range(N_CHUNKS):
                                sq = bw.tile([128, 512], F32R, tag="sqr",
                                             bufs=2, name="sq")
                                nc.vector.tensor_mul(
                                    sq[:], x[:, 512 * c:512 * (c + 1)],
                                    x[:, 512 * c:512 * (c + 1)])
                                sp = nps.tile([1, 512], F32, tag="ssq")
                                nc.tensor.matmul(sp[:], ones_col[:], sq[:],
                                                 start=True, stop=True)
                                srt = bw.tile([1, 512], F32, tag="srt",
                                              bufs=2, name="srt")
                                nc.scalar.activation(srt[:], sp[:], AF.Sqrt,
                                                     bias=eps_sb[:],
                                                     scale=1.0 / DH)
                                if is_q:
                                    rq = bw.tile([1, 512], F32R, tag="rq",
                                                 bufs=2, name="rq")
                                    with nc.allow_low_precision(
                                            reason="rstd f32r ample"):
                                        nc.vector.reciprocal(rq[:], srt[:])
                                    bp = bps.tile([128, 512], F32, tag="bcp")
                                    nc.tensor.matmul(bp[:], ones_row[:],
                                                     rq[:], start=True,
                                                     stop=True)
                                    nc.scalar.copy(
                                        bc[:, 512 * c:512 * (c + 1)], bp[:])
                                else:
                                    rk = bw.tile([1, 512], F32, tag="rk",
                                                 bufs=2, name="rk")
                                    nc.vector.reciprocal(rk[:], srt[:])
                                    nc.sync.dma_start(
                                        rk_d[4 * c:4 * (c + 1), :], rk[:])
                            if mt == 2:
                                nc.sync.dma_start(
                                    rkt[:],
                                    rk_d[:].rearrange("i p -> p i"))
                            # rope: roped = (x*A + shift64(x)*B) [* bc for q]
                            A_ = ropes["Aq"] if is_q else ropes["Ak"]
                            B_ = ropes["Bq"] if is_q else ropes["Bk"]
                            sh = bw.tile([128, S], F32, tag="shift",
                                         name="sh")
                            nc.sync.dma_start(sh[0:64, :], x[64:128, :])
                            nc.sync.dma_start(sh[64:128, :], x[0:64, :])
                            nc.vector.tensor_mul(sh[:], sh[:], B_[:])
                            tm = bw.tile([128, S], F32, tag="ropetmp",
                                         name="tm")
                            nc.vector.tensor_mul(tm[:], x[:], A_[:])
                            ro = pers.tile([128, S], BF16,
                                           tag=f"roped{b}{mt}",
                                           name=f"roped{b}{mt}")
                            if is_q:
                                nc.vector.tensor_add(tm[:], tm[:], sh[:])
                                nc.vector.tensor_mul(ro[:], tm[:], bc[:])
                            else:
                                nc.vector.tensor_add(ro[:], tm[:], sh[:])
                            roped[(b, mt)] = ro
                            if DEBUG and b == 0 and is_q and mt == 0:
                                nc.sync.dma_start(dbg["bc"].ap(), bc[:])
                    if DEBUG and b == 0:
                        nc.sync.dma_start(dbg["cn0"].ap(), cn[0][:])
                        nc.sync.dma_start(dbg["cn3"].ap(), cn[3][:])
                        nc.sync.dma_start(dbg["roped0"].ap(), roped[(0, 0)][:])
                        nc.sync.dma_start(dbg["roped2"].ap(), roped[(0, 2)][:])
                        nc.sync.dma_start(dbg["rstdkT"].ap(), rstdkT[0][:])
                        nc.sync.dma_start(dbg["vaug"].ap(), vaug[0][:])

            # ======================= attention =======================
            a2a_in = dram.tile([2048, 512], BF16, tag="a2ain", name="a2ain")
            a2a_out = dram.tile([2048, 512], BF16, tag="a2aout",
                                name="a2aout")

            with tc.tile_pool(name="scps", bufs=3, space="PSUM") as scps, \
                 tc.tile_pool(name="atps", bufs=4, space="PSUM") as atps, \
                 tc.tile_pool(name="apool", bufs=1) as apool:
                for b in range(B):
                    KT = roped[(b, 2)]
                    va = vaug[b]
                    rkt = rstdkT[b]
                    for h in range(2):
                        QT = roped[(b, h)]
                        for j in range(N_CHUNKS):
                            ptiles = []
                            for i in range(4 * j + 4):
                                r = i - 4 * j
                                off = 128 * max(r, 0)
                                ps = scps.tile([128, 512], F32, tag="sc",
                                               name="ps")
                                nc.tensor.matmul(
                                    ps[:, off:512],
                                    KT[:, 128 * i:128 * (i + 1)],
                                    QT[:, 512 * j + off:512 * (j + 1)],
                                    start=True, stop=True)
                                if r >= 0:
                                    nc.vector.tensor_add(
                                        ps[:, off:off + 128],
                                        ps[:, off:off + 128], mask_sb[:])
                                pt = apool.tile([128, 512], BF16, tag="p",
                                                bufs=18, name="pt")
                                nc.scalar.activation(
                                    pt[:, off:512], ps[:, off:512], AF.Exp,
                                    scale=rkt[:, i:i + 1])
                                if (DEBUG and b == 0 and h == 0
                                        and j == 0 and i == 0):
                                    nc.sync.dma_start(dbg["p00"].ap(), pt[:])
                                ptiles.append(pt)
                            for mp in range(4):
                                mg = 4 * j + mp
                                at = atps.tile([128, VSTR], F32, tag="at",
                                               name="at")
                                for i in range(mg + 1):
                                    nc.tensor.matmul(
                                        at[:, 0:129],
                                        ptiles[i][:, 128 * mp:128 * (mp + 1)],
                                        va[:, VSTR * i:VSTR * i + 129],
                                        start=(i == 0), stop=(i == mg))
                                rec = apool.tile([128, 1], F32, tag="rec",
                                                 bufs=3, name="rec")
                                nc.vector.reciprocal(rec[:], at[:, 128:129])
                                ab = apool.tile([128, 128], BF16, tag="ab",
                                                bufs=3, name="ab")
                                nc.vector.tensor_scalar_mul(
                                    ab[:], at[:, 0:128], rec[:])
                                if (DEBUG and b == 0 and h == 0
                                        and mg == 0):
                                    nc.sync.dma_start(dbg["ab0"].ap(), ab[:])
                                att = apool.tile([128, 128], BF16, tag="att",
                                                 bufs=3, name="att")
                                nc.sync.dma_start_transpose(att[:], ab[:])
                                rd = 4 * b + (mg // 4)
                                nc.sync.dma_start(
                                    a2a_in[256 * rd + 128 * h:
                                           256 * rd + 128 * (h + 1),
                                           128 * (mg % 4):128 * (mg % 4 + 1)],
                                    att[:])

            # ======================= all-to-all =======================
            nc.gpsimd.collective_compute(
                "AllToAll", ALU.bypass,
                replica_groups=[list(range(N_CORES))],
                ins=[a2a_in.opt()], outs=[a2a_out.opt()],
                cc_dim="Partition")

            if DEBUG:
                nc.sync.dma_start(dbg["a2ain"].ap(), a2a_in[:])
                nc.sync.dma_start(dbg["a2aout"].ap(), a2a_out[:])

            # ====================== out projection ====================
            with tc.tile_pool(name="opool", bufs=1) as opool, \
                 tc.tile_pool(name="ops", bufs=2, space="PSUM") as ops:
                aout = opool.tile([128, 16 * 512], BF16, tag="aout")
                nc.sync.dma_start(
                    aout[:].rearrange("p (k s) -> p k s", s=512),
                    a2a_out[:].rearrange("(k p) s -> p k s", p=128))
                av = aout[:].rearrange("p (k s) -> p k s", s=512)
                for n in range(4):
                    pso = [ops.tile([128, 512], F32, tag=f"o{m}",
                                    name=f"o{m}") for m in range(4)]
                    for k in range(16):
                        wo_t = opool.tile([128, 512], BF16, tag="wo", bufs=6,
                                          name="wo_t")
                        nc.sync.dma_start(
                            wo_t[:],
                            woT.ap()[128 * k:128 * (k + 1),
                                     512 * n:512 * (n + 1)])
                        for mp in range(4):
                            nc.tensor.matmul(
                                pso[mp][:],
                                av[:, k, 128 * mp:128 * (mp + 1)],
                                wo_t[:], start=(k == 0), stop=(k == 15))
                    for mp in range(4):
                        os_t = opool.tile([128, 512], F32, tag="osb", bufs=3,
                                          name="os_t")
                        nc.scalar.copy(os_t[:], pso[mp][:])
                        nc.sync.dma_start(
                            out.ap()[128 * mp:128 * (mp + 1),
                                     512 * n:512 * (n + 1)], os_t[:])

    nc.compile()
    return nc


_NC_CACHE = None


def _get_nc():
    global _NC_CACHE
    if _NC_CACHE is None:
        _NC_CACHE = _build()
    return _NC_CACHE


def _host_prep(inputs):
    hs = np.asarray(inputs["hidden_states"], dtype=np.float32)
    Wq = np.asarray(inputs["Wq"], dtype=np.float32)
    Wk = np.asarray(inputs["Wk"], dtype=np.float32)
    Wv = np.asarray(inputs["Wv"], dtype=np.float32)
    Wo = np.asarray(inputs["Wo"], dtype=np.float32)
    cqw = np.asarray(inputs["canon_q_w"], dtype=np.float32)
    ckw = np.asarray(inputs["canon_k_w"], dtype=np.float32)
    cvw = np.asarray(inputs["canon_v_w"], dtype=np.float32)
    qnw = np.asarray(inputs["q_norm_w"], dtype=np.float32)
    knw = np.asarray(inputs["k_norm_w"], dtype=np.float32)

    bf = ml_dtypes.bfloat16
    hsT = np.ascontiguousarray(
        np.concatenate([hs[0].T, hs[1].T], axis=1)).astype(bf)
    WqT, WkT, WvT = Wq.T, Wk.T, Wv.T
    woT = np.ascontiguousarray(Wo.T).astype(bf)

    inv_freq = 1.0 / (10000.0 ** (np.arange(0, DH, 2, dtype=np.float64) / DH))
    freqs = np.arange(S, dtype=np.float64)[:, None] * inv_freq
    emb = np.concatenate([freqs, freqs], axis=-1)
    cosT, sinT = np.cos(emb).T, np.sin(emb).T

    def make_rope(normw, scale):
        A = cosT * normw[:, None] * scale
        wswap = normw[(np.arange(DH) + 64) % DH]
        sign = np.where(np.arange(DH) < 64, -1.0, 1.0)
        Bc = sinT * wswap[:, None] * sign[:, None] * scale
        return (np.ascontiguousarray(A).astype(np.float32),
                np.ascontiguousarray(Bc).astype(np.float32))

    Aq, Bq = make_rope(qnw, SCALE)
    Ak, Bk = make_rope(knw, 1.0)

    p = np.arange(128)[:, None]
    f = np.arange(128)[None, :]
    maskd = np.where(p <= f, 0.0, NEG).astype(np.float32)

    in_maps = []
    for r in range(N_CORES):
        wTc = np.ascontiguousarray(np.concatenate(
            [WqT[:, 256 * r:256 * r + 256],
             WkT[:, 128 * r:128 * r + 128],
             WvT[:, 128 * r:128 * r + 128]], axis=1)).astype(bf)
        cwc = np.ascontiguousarray(np.concatenate(
            [cqw[256 * r:256 * r + 256],
             ckw[128 * r:128 * r + 128],
             cvw[128 * r:128 * r + 128]], axis=0)).astype(np.float32)
        in_maps.append({
            "hsT": hsT, "wT": wTc, "woT": woT, "cw": cwc,
            "ropeAq": Aq, "ropeBq": Bq, "ropeAk": Ak, "ropeBk": Bk,
            "maskd": maskd,
        })
    return in_maps


def kernel(**inputs):
    nc = _get_nc()
    in_maps = _host_prep(inputs)
    res = run_bass_kernel_spmd(nc, in_maps, core_ids=list(range(N_CORES)))
    full = np.empty((B, S, D), np.float32)
    for r in range(N_CORES):
        full[r // 4, 512 * (r % 4):512 * (r % 4 + 1), :] = res.results[r]["out"]
    return full



# revision 2
# speedup vs baseline: 1.2278x; 1.2278x over previous
"""Trainium2 Bass kernel for CanonCausalMultiheadAttn.

Sharding: tensor-parallel over heads across 8 cores (2 q-heads + 1 kv-head
per core), both batches replicated. Attention outputs are exchanged with two
head-split AllToAlls (h=0 fires while h=1 attention still computes); the
output projection contracts even-head dims first so it overlaps the second
collective.

Per-core pipeline (all shapes hardcoded for B=2, S=2048, D=2048):
  front-end fused per 512-token chunk: QKV proj (bf16 matmul) -> canon conv
  (DVE) -> qk rmsnorm via PE column-sum matmuls (rstd via fast approx
  reciprocal; q's rstd broadcast with a K=1 outer-product matmul, k's rstd
  applied later as the per-partition `scale` of the exp activation) -> RoPE
  (DVE, norm-weight & 1/sqrt(dh) folded into host cos/sin tables) ->
  causal attention with scores in [Sk, Sq] layout (bf16 matmul; no
  max-subtraction needed since |logit| <= sqrt(128) after qk-norm) ->
  exp (ACT, bf16 out) -> P@V with a ones-column appended to V giving row
  sums for free -> dual AllToAll -> output projection (bf16 matmul).
"""
import sys

sys.path.insert(0, '/opt/trn_rl_repo')

import numpy as np
import ml_dtypes

import concourse.bass as bass
import concourse.mybir as mybir
import concourse.tile as tile
from concourse import bacc
from concourse.bass_utils import run_bass_kernel_spmd

F32 = mybir.dt.float32
F32R = mybir.dt.float32r
BF16 = mybir.dt.bfloat16
AF = mybir.ActivationFunctionType
ALU = mybir.AluOpType

B, S, D = 2, 2048, 2048
NH, NKV, DH = 16, 8, 128
K_CONV = 4
EPS = 1e-6
SCALE = 1.0 / float(np.sqrt(DH))
NEG = -1e9
N_CORES = 8
N_CHUNKS = S // 512     # 512-wide chunks per batch
N_SKB = S // 128        # Sk blocks per batch
VSTR = 144              # V_aug stride per Sk block; 144*2B = 288B keeps each
                        # block 32B-aligned for the xbar DMA transpose


def _build():
    nc = bacc.Bacc("TRN2", target_bir_lowering=False, debug=False,
                   num_devices=N_CORES)

    hsT = nc.dram_tensor("hsT", [D, B * S], BF16, kind="ExternalInput")
    wT = nc.dram_tensor("wT", [D, 512], BF16, kind="ExternalInput")
    woT = nc.dram_tensor("woT", [D, D], BF16, kind="ExternalInput")
    cw = nc.dram_tensor("cw", [512, K_CONV], F32, kind="ExternalInput")
    ropeAq = nc.dram_tensor("ropeAq", [DH, S], F32, kind="ExternalInput")
    ropeBq = nc.dram_tensor("ropeBq", [DH, S], F32, kind="ExternalInput")
    ropeAk = nc.dram_tensor("ropeAk", [DH, S], F32, kind="ExternalInput")
    ropeBk = nc.dram_tensor("ropeBk", [DH, S], F32, kind="ExternalInput")
    maskd = nc.dram_tensor("maskd", [128, 128], F32, kind="ExternalInput")
    out = nc.dram_tensor("out", [512, D], F32, kind="ExternalOutput")

    with tile.TileContext(nc) as tc:
        with tc.tile_pool(name="const", bufs=1) as cpool, \
             tc.tile_pool(name="persist", bufs=1) as pers, \
             tc.tile_pool(name="dram", bufs=1, space="DRAM") as dram:

            # ---- constants ----
            ropes = {}
            for nm, t in (("Aq", ropeAq), ("Bq", ropeBq),
                          ("Ak", ropeAk), ("Bk", ropeBk)):
                rt = cpool.tile([DH, S], F32, tag=f"rope{nm}", name=f"rope{nm}")
                nc.sync.dma_start(rt[:], t.ap())
                ropes[nm] = rt
            mask_sb = cpool.tile([128, 128], F32, tag="mask")
            nc.sync.dma_start(mask_sb[:], maskd.ap())
            cw_sb = []
            for mt in range(4):
                t = cpool.tile([128, K_CONV], F32, tag=f"cw{mt}", name=f"cw{mt}")
                nc.sync.dma_start(t[:], cw.ap()[128 * mt:128 * mt + 128, :])
                cw_sb.append(t)
            ones_col_f = cpool.tile([128, 1], F32, tag="ocf")
            nc.vector.memset(ones_col_f[:], 1.0)
            ones_col = cpool.tile([128, 1], F32R, tag="oc")
            nc.scalar.copy(ones_col[:], ones_col_f[:])
            eps_sb = cpool.tile([1, 1], F32, tag="eps")
            nc.vector.memset(eps_sb[:], EPS)
            ones_row_f = cpool.tile([1, 128], F32, tag="orf")
            nc.vector.memset(ones_row_f[:], 1.0)
            ones_row = cpool.tile([1, 128], F32R, tag="or")
            nc.scalar.copy(ones_row[:], ones_row_f[:])
            s0_sb = []
            for mt in range(4):
                t = cpool.tile([128, 1], F32, tag=f"s0{mt}", name=f"s0{mt}")
                nc.vector.tensor_scalar_add(t[:], cw_sb[mt][:, 0:1], 1.0)
                s0_sb.append(t)
            # persistent weight tile: wT [2048, 512] -> [128, 16, 512]
            wt_sb = pers.tile([128, 16 * 512], BF16, tag="wt", name="wt_sb")
            nc.sync.dma_start(
                wt_sb[:].rearrange("p (k s) -> p k s", s=512),
                wT.ap().rearrange("(k p) s -> p k s", p=128))
            wt = wt_sb[:].rearrange("p (k s) -> p k s", s=512)

            # persistent per-(b,mt) outputs of the front-end
            roped = {}   # (b, mt<3) -> [128, S] bf16
            vaug = {}    # b -> [128, N_SKB*VSTR] bf16
            rstdkT = {}  # b -> [128, N_SKB] f32 (k rstd, transposed per block)
            for b in range(B):
                vaug[b] = pers.tile([128, N_SKB * VSTR], BF16, tag=f"vaug{b}",
                                    name=f"vaug{b}")
                rstdkT[b] = pers.tile([128, N_SKB], F32, tag=f"rstdkT{b}",
                                      name=f"rstdkT{b}")
                for mt in range(3):
                    roped[(b, mt)] = pers.tile([128, S], BF16,
                                               tag=f"roped{b}{mt}",
                                               name=f"roped{b}{mt}")

            # ============ front-end: QKV + canon + norm + rope ============
            # fused per 512-token chunk so DVE/ACT work pipelines with the
            # next chunk's matmuls.
            with tc.tile_pool(name="fwork", bufs=1) as fw, \
                 tc.tile_pool(name="qps", bufs=1, space="PSUM") as qps, \
                 tc.tile_pool(name="nps", bufs=2, space="PSUM") as nps, \
                 tc.tile_pool(name="bps", bufs=2, space="PSUM") as bps:
                rk_d = {}
                for b in range(B):
                    rk_d[b] = dram.tile([N_SKB, 128], F32, tag=f"rkd{b}",
                                        name=f"rk_d{b}")
                for b in range(B):
                    prev_raw = {}
                    for n in range(N_CHUNKS):
                        lo = 512 * n
                        hs_sb = fw.tile([128, 16 * 512], BF16, tag="hschunk",
                                        bufs=2, name="hs_sb")
                        nc.sync.dma_start(
                            hs_sb[:].rearrange("p (k s) -> p k s", s=512),
                            hsT.ap()[:, b * S + lo:b * S + lo + 512]
                            .rearrange("(k p) s -> p k s", p=128))
                        hv = hs_sb[:].rearrange("p (k s) -> p k s", s=512)
                        psums = [qps.tile([128, 512], F32, tag=f"qk{mt}",
                                          name=f"qk{mt}") for mt in range(4)]
                        for k in range(16):
                            for mt in range(4):
                                nc.tensor.matmul(
                                    psums[mt][:],
                                    wt[:, k, 128 * mt:128 * (mt + 1)],
                                    hv[:, k, :],
                                    start=(k == 0), stop=(k == 15))
                        for mt in range(4):
                            is_v = mt == 3
                            is_q = mt < 2
                            raw_c = fw.tile([128, 512], F32, tag=f"rawc{mt}",
                                            bufs=2, name=f"rawc{mt}")
                            nc.scalar.copy(raw_c[:], psums[mt][:])
                            # canon conv chunk (residual folded into s0)
                            c = fw.tile([128, 512], BF16 if is_v else F32,
                                        tag=f"cn{mt}", bufs=2, name=f"cn{mt}")
                            nc.vector.tensor_scalar_mul(
                                c[:], raw_c[:], s0_sb[mt][:])
                            for k in range(1, K_CONV):
                                nc.vector.scalar_tensor_tensor(
                                    c[:, k:512], raw_c[:, 0:512 - k],
                                    cw_sb[mt][:, k:k + 1],
                                    c[:, k:512], ALU.mult, ALU.add)
                                if n > 0:
                                    nc.vector.scalar_tensor_tensor(
                                        c[:, 0:k],
                                        prev_raw[mt][:, 512 - k:512],
                                        cw_sb[mt][:, k:k + 1],
                                        c[:, 0:k], ALU.mult, ALU.add)
                            prev_raw[mt] = raw_c
                            if is_v:
                                # transpose V chunk into V_aug blocks
                                va = vaug[b]
                                for i in range(4):
                                    blk = 4 * n + i
                                    nc.sync.dma_start_transpose(
                                        va[:, VSTR * blk:VSTR * blk + 128],
                                        c[:, 128 * i:128 * (i + 1)])
                                continue
                            # ---- rmsnorm rstd over dh (partition dim) ----
                            sq = fw.tile([128, 512], F32R, tag="sqr", bufs=2,
                                         name="sq")
                            nc.vector.tensor_mul(sq[:], c[:], c[:])
                            sp = nps.tile([1, 512], F32, tag="ssq")
                            nc.tensor.matmul(sp[:], ones_col[:], sq[:],
                                             start=True, stop=True)
                            srt = fw.tile([1, 512], F32, tag="srt", bufs=2,
                                          name="srt")
                            nc.scalar.activation(srt[:], sp[:], AF.Sqrt,
                                                 bias=eps_sb[:],
                                                 scale=1.0 / DH)
                            bc = None
                            if is_q:
                                rq = fw.tile([1, 512], F32, tag="rq", bufs=2,
                                             name="rq")
                                nc.vector.reciprocal_approx_fast(rq[:], srt[:])
                                rqr = fw.tile([1, 512], F32R, tag="rqr",
                                              bufs=2, name="rqr")
                                nc.scalar.copy(rqr[:], rq[:])
                                bp = bps.tile([128, 512], F32, tag="bcp")
                                nc.tensor.matmul(bp[:], ones_row[:], rqr[:],
                                                 start=True, stop=True)
                                bc = fw.tile([128, 512], F32, tag="bc",
                                             bufs=2, name="bc")
                                nc.scalar.copy(bc[:], bp[:])
                            else:
                                rk = fw.tile([1, 512], F32, tag="rk", bufs=2,
                                             name="rk")
                                nc.vector.reciprocal_approx_fast(rk[:], srt[:])
                                nc.sync.dma_start(
                                    rk_d[b][4 * n:4 * (n + 1), :], rk[:])
                            # ---- rope chunk ----
                            A_ = ropes["Aq"] if is_q else ropes["Ak"]
                            B_ = ropes["Bq"] if is_q else ropes["Bk"]
                            sh = fw.tile([128, 512], F32, tag="shift",
                                         bufs=2, name="sh")
                            nc.sync.dma_start(sh[0:64, :], c[64:128, :])
                            nc.sync.dma_start(sh[64:128, :], c[0:64, :])
                            nc.vector.tensor_mul(sh[:], sh[:],
                                                 B_[:, lo:lo + 512])
                            tm = fw.tile([128, 512], F32, tag="ropetmp",
                                         bufs=2, name="tm")
                            nc.vector.tensor_mul(tm[:], c[:],
                                                 A_[:, lo:lo + 512])
                            ro = roped[(b, mt)]
                            if is_q:
                                nc.vector.tensor_add(tm[:], tm[:], sh[:])
                                nc.vector.tensor_mul(ro[:, lo:lo + 512],
                                                     tm[:], bc[:])
                            else:
                                nc.vector.tensor_add(ro[:, lo:lo + 512],
                                                     tm[:], sh[:])
                    # ones in the V_aug sum column for this batch
                    nc.vector.memset(
                        vaug[b][:].rearrange("p (i c) -> p i c",
                                             c=VSTR)[:, :, 128:129], 1.0)
                    nc.sync.dma_start(rstdkT[b][:],
                                      rk_d[b][:].rearrange("i p -> p i"))

            # ======================= attention =======================
            # h-major: both batches' h=0 heads first, then AllToAll #0 can
            # fly while h=1 attention computes.
            a2a_in = [dram.tile([1024, 512], BF16, tag=f"a2ain{h}",
                                name=f"a2ain{h}") for h in range(2)]
            a2a_out = [dram.tile([1024, 512], BF16, tag=f"a2aout{h}",
                                 name=f"a2aout{h}") for h in range(2)]

            with tc.tile_pool(name="scps", bufs=3, space="PSUM") as scps, \
                 tc.tile_pool(name="atps", bufs=3, space="PSUM") as atps, \
                 tc.tile_pool(name="apool", bufs=1) as apool:
                for h in range(2):
                    for b in range(B):
                        KT = roped[(b, 2)]
                        va = vaug[b]
                        rkt = rstdkT[b]
                        QT = roped[(b, h)]
                        for j in range(N_CHUNKS):
                            ptiles = []
                            for i in range(4 * j + 4):
                                r = i - 4 * j
                                off = 128 * max(r, 0)
                                ps = scps.tile([128, 512], F32, tag="sc",
                                               name="ps")
                                nc.tensor.matmul(
                                    ps[:, off:512],
                                    KT[:, 128 * i:128 * (i + 1)],
                                    QT[:, 512 * j + off:512 * (j + 1)],
                                    start=True, stop=True)
                                if r >= 0:
                                    nc.vector.tensor_add(
                                        ps[:, off:off + 128],
                                        ps[:, off:off + 128], mask_sb[:])
                                pt = apool.tile([128, 512], BF16, tag="p",
                                                bufs=18, name="pt")
                                nc.scalar.activation(
                                    pt[:, off:512], ps[:, off:512], AF.Exp,
                                    scale=rkt[:, i:i + 1])
                                ptiles.append(pt)
                            for mp in range(4):
                                mg = 4 * j + mp
                                at = atps.tile([128, VSTR], F32, tag="at",
                                               name="at")
                                for i in range(mg + 1):
                                    nc.tensor.matmul(
                                        at[:, 0:129],
                                        ptiles[i][:, 128 * mp:128 * (mp + 1)],
                                        va[:, VSTR * i:VSTR * i + 129],
                                        start=(i == 0), stop=(i == mg))
                                rec = apool.tile([128, 1], F32, tag="rec",
                                                 bufs=3, name="rec")
                                nc.vector.reciprocal_approx_fast(
                                    rec[:], at[:, 128:129])
                                ab = apool.tile([128, 128], BF16, tag="ab",
                                                bufs=3, name="ab")
                                nc.vector.tensor_scalar_mul(
                                    ab[:], at[:, 0:128], rec[:])
                                att = apool.tile([128, 128], BF16, tag="att",
                                                 bufs=3, name="att")
                                nc.sync.dma_start_transpose(att[:], ab[:])
                                rd = 4 * b + j
                                nc.sync.dma_start(
                                    a2a_in[h][128 * rd:128 * (rd + 1),
                                              128 * mp:128 * (mp + 1)],
                                    att[:])
                    # ---- AllToAll for this head (overlaps h=1 compute) ----
                    nc.gpsimd.collective_compute(
                        "AllToAll", ALU.bypass,
                        replica_groups=[list(range(N_CORES))],
                        ins=[a2a_in[h].opt()], outs=[a2a_out[h].opt()],
                        cc_dim="Partition")

            # ====================== out projection ====================
            # contract even-head dims (a2a#0 payload) first so those matmuls
            # overlap the second collective.
            with tc.tile_pool(name="opool", bufs=1) as opool, \
                 tc.tile_pool(name="ops", bufs=1, space="PSUM") as ops:
                aout = {}
                for h in range(2):
                    ao = opool.tile([128, 8 * 512], BF16, tag=f"aout{h}",
                                    name=f"aout{h}")
                    for k in range(8):
                        nc.sync.dma_start(
                            ao[:, 512 * k:512 * (k + 1)],
                            a2a_out[h][128 * k:128 * (k + 1), :])
                    aout[h] = ao
                for ng in range(2):     # n-pair groups: (0,1) then (2,3)
                    pso = {}
                    for dn in range(2):
                        for mp in range(4):
                            pso[(dn, mp)] = ops.tile(
                                [128, 512], F32, tag=f"o{dn}{mp}",
                                name=f"o{dn}{mp}")
                    for h in range(2):
                        av = aout[h][:].rearrange("p (k s) -> p k s", s=512)
                        for dn in range(2):
                            n = 2 * ng + dn
                            for k in range(8):
                                wo_t = opool.tile([128, 512], BF16, tag="wo",
                                                  bufs=6, name="wo_t")
                                nc.sync.dma_start(
                                    wo_t[:],
                                    woT.ap()[1024 * h + 128 * k:
                                             1024 * h + 128 * (k + 1),
                                             512 * n:512 * (n + 1)])
                                for mp in range(4):
                                    nc.tensor.matmul(
                                        pso[(dn, mp)][:],
                                        av[:, k, 128 * mp:128 * (mp + 1)],
                                        wo_t[:],
                                        start=(h == 0 and k == 0),
                                        stop=(h == 1 and k == 7))
                    for dn in range(2):
                        n = 2 * ng + dn
                        for mp in range(4):
                            os_t = opool.tile([128, 512], F32, tag="osb",
                                              bufs=4, name="os_t")
                            nc.scalar.copy(os_t[:], pso[(dn, mp)][:])
                            nc.sync.dma_start(
                                out.ap()[128 * mp:128 * (mp + 1),
                                         512 * n:512 * (n + 1)], os_t[:])

    nc.compile()
    return nc


_NC_CACHE = None


def _get_nc():
    global _NC_CACHE
    if _NC_CACHE is None:
        _NC_CACHE = _build()
    return _NC_CACHE


def _host_prep(inputs):
    hs = np.asarray(inputs["hidden_states"], dtype=np.float32)
    Wq = np.asarray(inputs["Wq"], dtype=np.float32)
    Wk = np.asarray(inputs["Wk"], dtype=np.float32)
    Wv = np.asarray(inputs["Wv"], dtype=np.float32)
    Wo = np.asarray(inputs["Wo"], dtype=np.float32)
    cqw = np.asarray(inputs["canon_q_w"], dtype=np.float32)
    ckw = np.asarray(inputs["canon_k_w"], dtype=np.float32)
    cvw = np.asarray(inputs["canon_v_w"], dtype=np.float32)
    qnw = np.asarray(inputs["q_norm_w"], dtype=np.float32)
    knw = np.asarray(inputs["k_norm_w"], dtype=np.float32)

    bf = ml_dtypes.bfloat16
    hsT = np.ascontiguousarray(
        np.concatenate([hs[0].T, hs[1].T], axis=1)).astype(bf)
    WqT, WkT, WvT = Wq.T, Wk.T, Wv.T
    # Wo^T with even-head (h=0 per core) row-blocks first, then odd
    woT_full = Wo.T
    blocks = woT_full.reshape(16, 128, D)
    woT = np.ascontiguousarray(
        np.concatenate([blocks[0::2], blocks[1::2]], axis=0).reshape(D, D)
    ).astype(bf)

    inv_freq = 1.0 / (10000.0 ** (np.arange(0, DH, 2, dtype=np.float64) / DH))
    freqs = np.arange(S, dtype=np.float64)[:, None] * inv_freq
    emb = np.concatenate([freqs, freqs], axis=-1)
    cosT, sinT = np.cos(emb).T, np.sin(emb).T

    def make_rope(normw, scale):
        A = cosT * normw[:, None] * scale
        wswap = normw[(np.arange(DH) + 64) % DH]
        sign = np.where(np.arange(DH) < 64, -1.0, 1.0)
        Bc = sinT * wswap[:, None] * sign[:, None] * scale
        return (np.ascontiguousarray(A).astype(np.float32),
                np.ascontiguousarray(Bc).astype(np.float32))

    Aq, Bq = make_rope(qnw, SCALE)
    Ak, Bk = make_rope(knw, 1.0)

    p = np.arange(128)[:, None]
    f = np.arange(128)[None, :]
    maskd = np.where(p <= f, 0.0, NEG).astype(np.float32)

    in_maps = []
    for r in range(N_CORES):
        wTc = np.ascontiguousarray(np.concatenate(
            [WqT[:, 256 * r:256 * r + 256],
             WkT[:, 128 * r:128 * r + 128],
             WvT[:, 128 * r:128 * r + 128]], axis=1)).astype(bf)
        cwc = np.ascontiguousarray(np.concatenate(
            [cqw[256 * r:256 * r + 256],
             ckw[128 * r:128 * r + 128],
             cvw[128 * r:128 * r + 128]], axis=0)).astype(np.float32)
        in_maps.append({
            "hsT": hsT, "wT": wTc, "woT": woT, "cw": cwc,
            "ropeAq": Aq, "ropeBq": Bq, "ropeAk": Ak, "ropeBk": Bk,
            "maskd": maskd,
        })
    return in_maps


def kernel(**inputs):
    nc = _get_nc()
    in_maps = _host_prep(inputs)
    res = run_bass_kernel_spmd(nc, in_maps, core_ids=list(range(N_CORES)))
    full = np.empty((B, S, D), np.float32)
    for r in range(N_CORES):
        full[r // 4, 512 * (r % 4):512 * (r % 4 + 1), :] = res.results[r]["out"]
    return full


# revision 15
# speedup vs baseline: 1.3395x; 1.0910x over previous
"""Trainium2 Bass kernel for CanonCausalMultiheadAttn.

Sharding: tensor-parallel over heads across 8 cores (2 q-heads + 1 kv-head
per core), both batches replicated. Attention outputs are exchanged with two
head-split AllToAlls (h=0 fires while h=1 attention still computes); the
output projection contracts even-head dims first so it overlaps the second
collective.

Per-core pipeline (all shapes hardcoded for B=2, S=2048, D=2048):
  front-end fused per 512-token chunk: QKV proj (bf16 matmul) -> canon conv
  (DVE) -> qk rmsnorm via PE column-sum matmuls (rstd via fast approx
  reciprocal; q's rstd broadcast with a K=1 outer-product matmul, k's rstd
  applied later as the per-partition `scale` of the exp activation) -> RoPE
  (DVE, norm-weight & 1/sqrt(dh) folded into host cos/sin tables) ->
  causal attention with scores in [Sk, Sq] layout (bf16 matmul; no
  max-subtraction needed since |logit| <= sqrt(128) after qk-norm) ->
  exp (ACT, bf16 out) -> P@V with a ones-column appended to V giving row
  sums for free -> dual AllToAll -> output projection (bf16 matmul).
"""
import sys

sys.path.insert(0, '/opt/trn_rl_repo')

import numpy as np
import ml_dtypes

import concourse.bass as bass
import concourse.mybir as mybir
import concourse.tile as tile
from concourse import bacc
from concourse.bass_utils import run_bass_kernel_spmd

F32 = mybir.dt.float32
F32R = mybir.dt.float32r
BF16 = mybir.dt.bfloat16
AF = mybir.ActivationFunctionType
ALU = mybir.AluOpType

B, S, D = 2, 2048, 2048
NH, NKV, DH = 16, 8, 128
K_CONV = 4
EPS = 1e-6
SCALE = 1.0 / float(np.sqrt(DH))
NEG = -1e9
N_CORES = 8
N_CHUNKS = S // 512     # 512-wide chunks per batch
N_SKB = S // 128        # Sk blocks per batch
VSTR = 144              # V_aug stride per Sk block; 144*2B = 288B keeps each
                        # block 32B-aligned for the xbar DMA transpose


def _build():
    nc = bacc.Bacc("TRN2", target_bir_lowering=False, debug=False,
                   num_devices=N_CORES)

    hsT = nc.dram_tensor("hsT", [D, B * S], BF16, kind="ExternalInput")
    wT = nc.dram_tensor("wT", [D, 512], BF16, kind="ExternalInput")
    woT = nc.dram_tensor("woT", [D, D], BF16, kind="ExternalInput")
    cw = nc.dram_tensor("cw", [512, K_CONV], F32, kind="ExternalInput")
    ropeAq = nc.dram_tensor("ropeAq", [DH, S], BF16, kind="ExternalInput")
    ropeBq = nc.dram_tensor("ropeBq", [DH, S], BF16, kind="ExternalInput")
    ropeAk = nc.dram_tensor("ropeAk", [DH, S], BF16, kind="ExternalInput")
    ropeBk = nc.dram_tensor("ropeBk", [DH, S], BF16, kind="ExternalInput")
    maskd = nc.dram_tensor("maskd", [128, 128], F32, kind="ExternalInput")
    out = nc.dram_tensor("out", [512, D], F32, kind="ExternalOutput")

    with tile.TileContext(nc) as tc:
        with tc.tile_pool(name="const", bufs=1) as cpool, \
             tc.tile_pool(name="persist", bufs=1) as pers, \
             tc.tile_pool(name="dram", bufs=1, space="DRAM") as dram:

            # ---- constants ----
            # weight tile first: the first chunk's matmuls need it before
            # anything else, so its DMA leads the queue.
            wt_sb = pers.tile([128, 16 * 512], BF16, tag="wt", name="wt_sb")
            nc.sync.dma_start(
                wt_sb[:].rearrange("p (k s) -> p k s", s=512),
                wT.ap().rearrange("(k p) s -> p k s", p=128))
            wt = wt_sb[:].rearrange("p (k s) -> p k s", s=512)
            ropes = {}
            for nm, t in (("Aq", ropeAq), ("Bq", ropeBq),
                          ("Ak", ropeAk), ("Bk", ropeBk)):
                ropes[nm] = cpool.tile([DH, S], BF16, tag=f"rope{nm}",
                                       name=f"rope{nm}")
            mask_sb = cpool.tile([128, 128], F32, tag="mask")
            nc.sync.dma_start(mask_sb[:], maskd.ap())
            cw_sb = []
            for mt in range(4):
                t = cpool.tile([128, K_CONV], F32, tag=f"cw{mt}", name=f"cw{mt}")
                nc.sync.dma_start(t[:], cw.ap()[128 * mt:128 * mt + 128, :])
                cw_sb.append(t)
            ones_col_f = cpool.tile([128, 1], F32, tag="ocf")
            nc.vector.memset(ones_col_f[:], 1.0)
            ones_col = cpool.tile([128, 1], F32R, tag="oc")
            nc.scalar.copy(ones_col[:], ones_col_f[:])
            eps_sb = cpool.tile([1, 1], F32, tag="eps")
            nc.vector.memset(eps_sb[:], EPS)
            ones_row_f = cpool.tile([1, 128], F32, tag="orf")
            nc.vector.memset(ones_row_f[:], 1.0)
            ones_row = cpool.tile([1, 128], F32R, tag="or")
            nc.scalar.copy(ones_row[:], ones_row_f[:])
            s0_sb = []
            for mt in range(4):
                t = cpool.tile([128, 1], F32, tag=f"s0{mt}", name=f"s0{mt}")
                nc.vector.tensor_scalar_add(t[:], cw_sb[mt][:, 0:1], 1.0)
                s0_sb.append(t)
            # even-head half of Wo, preloaded during attention so the first
            # half of the output projection can overlap the second AllToAll
            wo_even = pers.tile([128, 8 * 4 * 512], BF16, tag="woe",
                                name="wo_even")

            # persistent per-(b,mt) outputs of the front-end
            roped = {}   # (b, mt<3) -> [128, S] bf16
            vaug = {}    # b -> [128, N_SKB*VSTR] bf16
            rstdkT = {}  # b -> [128, N_SKB] f32 (k rstd, transposed per block)
            for b in range(B):
                vaug[b] = pers.tile([128, N_SKB * VSTR], BF16, tag=f"vaug{b}",
                                    name=f"vaug{b}")
                rstdkT[b] = pers.tile([128, N_SKB], F32, tag=f"rstdkT{b}",
                                      name=f"rstdkT{b}")
                for mt in range(3):
                    roped[(b, mt)] = pers.tile([128, S], BF16,
                                               tag=f"roped{b}{mt}",
                                               name=f"roped{b}{mt}")

            # ============ front-end: QKV + canon + norm + rope ============
            # fused per 512-token chunk so DVE/ACT work pipelines with the
            # next chunk's matmuls.
            with tc.tile_pool(name="fwork", bufs=1) as fw, \
                 tc.tile_pool(name="qps", bufs=1, space="PSUM") as qps, \
                 tc.tile_pool(name="nps", bufs=2, space="PSUM") as nps, \
                 tc.tile_pool(name="bps", bufs=2, space="PSUM") as bps:
                rk_d = {}
                for b in range(B):
                    rk_d[b] = dram.tile([N_SKB, 128], F32, tag=f"rkd{b}",
                                        name=f"rk_d{b}")
                for b in range(B):
                    prev_raw = {}
                    for n in range(N_CHUNKS):
                        lo = 512 * n
                        hs_sb = fw.tile([128, 16 * 512], BF16, tag="hschunk",
                                        bufs=2, name="hs_sb")
                        nc.sync.dma_start(
                            hs_sb[:].rearrange("p (k s) -> p k s", s=512),
                            hsT.ap()[:, b * S + lo:b * S + lo + 512]
                            .rearrange("(k p) s -> p k s", p=128))
                        if b == 0 and n == 0:
                            # rope tables: enqueue behind the first chunk's
                            # activations so they don't delay the first matmul
                            for nm, t in (("Aq", ropeAq), ("Bq", ropeBq),
                                          ("Ak", ropeAk), ("Bk", ropeBk)):
                                nc.sync.dma_start(ropes[nm][:], t.ap())
                        hv = hs_sb[:].rearrange("p (k s) -> p k s", s=512)
                        psums = [qps.tile([128, 512], F32, tag=f"qk{mt}",
                                          name=f"qk{mt}") for mt in range(4)]
                        for k in range(16):
                            for mt in range(4):
                                nc.tensor.matmul(
                                    psums[mt][:],
                                    wt[:, k, 128 * mt:128 * (mt + 1)],
                                    hv[:, k, :],
                                    start=(k == 0), stop=(k == 15))
                        for mt in range(4):
                            is_v = mt == 3
                            is_q = mt < 2
                            raw_c = fw.tile([128, 512], BF16, tag=f"rawc{mt}",
                                            bufs=2, name=f"rawc{mt}")
                            nc.scalar.copy(raw_c[:], psums[mt][:])
                            # canon conv chunk (residual folded into s0)
                            c = fw.tile([128, 512], BF16,
                                        tag=f"cn{mt}", bufs=2, name=f"cn{mt}")
                            nc.vector.tensor_scalar_mul(
                                c[:], raw_c[:], s0_sb[mt][:])
                            for k in range(1, K_CONV):
                                nc.vector.scalar_tensor_tensor(
                                    c[:, k:512], raw_c[:, 0:512 - k],
                                    cw_sb[mt][:, k:k + 1],
                                    c[:, k:512], ALU.mult, ALU.add)
                                if n > 0:
                                    nc.vector.scalar_tensor_tensor(
                                        c[:, 0:k],
                                        prev_raw[mt][:, 512 - k:512],
                                        cw_sb[mt][:, k:k + 1],
                                        c[:, 0:k], ALU.mult, ALU.add)
                            prev_raw[mt] = raw_c
                            if is_v:
                                # transpose V chunk into V_aug blocks
                                va = vaug[b]
                                for i in range(4):
                                    blk = 4 * n + i
                                    nc.sync.dma_start_transpose(
                                        va[:, VSTR * blk:VSTR * blk + 128],
                                        c[:, 128 * i:128 * (i + 1)])
                                continue
                            # ---- rmsnorm rstd over dh (partition dim) ----
                            sq = fw.tile([128, 512], F32R, tag="sqr", bufs=2,
                                         name="sq")
                            nc.vector.tensor_mul(sq[:], c[:], c[:])
                            sp = nps.tile([1, 512], F32, tag="ssq")
                            nc.tensor.matmul(sp[:], ones_col[:], sq[:],
                                             start=True, stop=True)
                            srt = fw.tile([1, 512], F32, tag="srt", bufs=2,
                                          name="srt")
                            nc.scalar.activation(srt[:], sp[:], AF.Sqrt,
                                                 bias=eps_sb[:],
                                                 scale=1.0 / DH)
                            bc = None
                            if is_q:
                                rq = fw.tile([1, 512], F32, tag="rq", bufs=2,
                                             name="rq")
                                nc.vector.reciprocal_approx_fast(rq[:], srt[:])
                                rqr = fw.tile([1, 512], F32R, tag="rqr",
                                              bufs=2, name="rqr")
                                nc.scalar.copy(rqr[:], rq[:])
                                bp = bps.tile([128, 512], F32, tag="bcp")
                                nc.tensor.matmul(bp[:], ones_row[:], rqr[:],
                                                 start=True, stop=True)
                                bc = fw.tile([128, 512], F32, tag="bc",
                                             bufs=2, name="bc")
                                nc.scalar.copy(bc[:], bp[:])
                            else:
                                rk = fw.tile([1, 512], F32, tag="rk", bufs=2,
                                             name="rk")
                                nc.vector.reciprocal_approx_fast(rk[:], srt[:])
                                nc.sync.dma_start(
                                    rk_d[b][4 * n:4 * (n + 1), :], rk[:])
                            # ---- rope chunk ----
                            A_ = ropes["Aq"] if is_q else ropes["Ak"]
                            B_ = ropes["Bq"] if is_q else ropes["Bk"]
                            sh = fw.tile([128, 512], BF16, tag="shift",
                                         bufs=2, name="sh")
                            nc.sync.dma_start(sh[0:64, :], c[64:128, :])
                            nc.sync.dma_start(sh[64:128, :], c[0:64, :])
                            nc.vector.tensor_mul(sh[:], sh[:],
                                                 B_[:, lo:lo + 512])
                            tm = fw.tile([128, 512], BF16, tag="ropetmp",
                                         bufs=2, name="tm")
                            nc.vector.tensor_mul(tm[:], c[:],
                                                 A_[:, lo:lo + 512])
                            ro = roped[(b, mt)]
                            if is_q:
                                nc.vector.tensor_add(tm[:], tm[:], sh[:])
                                nc.vector.tensor_mul(ro[:, lo:lo + 512],
                                                     tm[:], bc[:])
                            else:
                                nc.vector.tensor_add(ro[:, lo:lo + 512],
                                                     tm[:], sh[:])
                    # ones in the V_aug sum column for this batch
                    nc.vector.memset(
                        vaug[b][:].rearrange("p (i c) -> p i c",
                                             c=VSTR)[:, :, 128:129], 1.0)
                    nc.sync.dma_start(rstdkT[b][:],
                                      rk_d[b][:].rearrange("i p -> p i"))

            # ======================= attention =======================
            # h-major: both batches' h=0 heads first, then AllToAll #0 can
            # fly while h=1 attention computes.
            a2a_in = [dram.tile([1024, 512], BF16, tag=f"a2ain{h}",
                                name=f"a2ain{h}") for h in range(2)]
            a2a_out = [dram.tile([1024, 512], BF16, tag=f"a2aout{h}",
                                 name=f"a2aout{h}") for h in range(2)]

            with tc.tile_pool(name="scps", bufs=3, space="PSUM") as scps, \
                 tc.tile_pool(name="atps", bufs=3, space="PSUM") as atps, \
                 tc.tile_pool(name="apool", bufs=1) as apool:
                for h in range(2):
                    if h == 1:
                        # prefetch the even-head Wo half while h=1 attention
                        # computes (sits behind h=0's DMAs in the queue)
                        nc.sync.dma_start(
                            wo_even[:].rearrange("p (k s) -> p k s", s=2048),
                            woT.ap()[0:1024, :]
                            .rearrange("(k p) s -> p k s", p=128))
                    for b in range(B):
                        KT = roped[(b, 2)]
                        va = vaug[b]
                        rkt = rstdkT[b]
                        QT = roped[(b, h)]
                        for j in range(N_CHUNKS):
                            ptiles = []
                            for i in range(4 * j + 4):
                                r = i - 4 * j
                                off = 128 * max(r, 0)
                                ps = scps.tile([128, 512], F32, tag="sc",
                                               name="ps")
                                nc.tensor.matmul(
                                    ps[:, off:512],
                                    KT[:, 128 * i:128 * (i + 1)],
                                    QT[:, 512 * j + off:512 * (j + 1)],
                                    start=True, stop=True)
                                if r >= 0:
                                    nc.vector.tensor_add(
                                        ps[:, off:off + 128],
                                        ps[:, off:off + 128], mask_sb[:])
                                pt = apool.tile([128, 512], BF16, tag="p",
                                                bufs=18, name="pt")
                                nc.scalar.activation(
                                    pt[:, off:512], ps[:, off:512], AF.Exp,
                                    scale=rkt[:, i:i + 1])
                                ptiles.append(pt)
                            for mp in range(4):
                                mg = 4 * j + mp
                                at = atps.tile([128, VSTR], F32, tag="at",
                                               name="at")
                                for i in range(mg + 1):
                                    nc.tensor.matmul(
                                        at[:, 0:129],
                                        ptiles[i][:, 128 * mp:128 * (mp + 1)],
                                        va[:, VSTR * i:VSTR * i + 129],
                                        start=(i == 0), stop=(i == mg))
                                rec = apool.tile([128, 1], F32, tag="rec",
                                                 bufs=6, name="rec")
                                nc.vector.reciprocal_approx_fast(
                                    rec[:], at[:, 128:129])
                                ab = apool.tile([128, 128], BF16, tag="ab",
                                                bufs=6, name="ab")
                                nc.vector.tensor_scalar_mul(
                                    ab[:], at[:, 0:128], rec[:])
                                att = apool.tile([128, 128], BF16, tag="att",
                                                 bufs=6, name="att")
                                nc.sync.dma_start_transpose(att[:], ab[:])
                                rd = 4 * b + j
                                nc.sync.dma_start(
                                    a2a_in[h][128 * rd:128 * (rd + 1),
                                              128 * mp:128 * (mp + 1)],
                                    att[:])

            # Both collectives are enqueued after all attention DMAs: their
            # completion-waits live in the Sync queue and would otherwise
            # fence every later DMA. The gpsimd trigger for #0 only waits on
            # a2a_in[0] writes, so it still fires as soon as h=0 is done and
            # the exchange overlaps h=1 compute. The aout[0] gather is
            # enqueued between the two so it only sits behind fence #0.
            aout = {}
            for h in range(2):
                aout[h] = pers.tile([128, 8 * 512], BF16, tag=f"aout{h}",
                                    name=f"aout{h}")
            for h in range(2):
                nc.gpsimd.collective_compute(
                    "AllToAll", ALU.bypass,
                    replica_groups=[list(range(N_CORES))],
                    ins=[a2a_in[h].opt()], outs=[a2a_out[h].opt()],
                    cc_dim="Partition")
                for k in range(8):
                    nc.sync.dma_start(
                        aout[h][:, 512 * k:512 * (k + 1)],
                        a2a_out[h][128 * k:128 * (k + 1), :])

            # ====================== out projection ====================
            # contract even-head dims (a2a#0 payload) first so those matmuls
            # overlap the second collective.
            with tc.tile_pool(name="opool", bufs=1) as opool, \
                 tc.tile_pool(name="ops", bufs=1, space="PSUM") as ops:
                for ng in range(2):     # n-pair groups: (0,1) then (2,3)
                    pso = {}
                    for dn in range(2):
                        for mp in range(4):
                            pso[(dn, mp)] = ops.tile(
                                [128, 512], F32, tag=f"o{dn}{mp}",
                                name=f"o{dn}{mp}")
                    woe = wo_even[:].rearrange("p (k s) -> p k s", s=2048)
                    for h in range(2):
                        av = aout[h][:].rearrange("p (k s) -> p k s", s=512)
                        for dn in range(2):
                            n = 2 * ng + dn
                            for k in range(8):
                                if h == 0:
                                    wo_t = woe[:, k, 512 * n:512 * (n + 1)]
                                else:
                                    wo_s = opool.tile([128, 512], BF16,
                                                      tag="wo", bufs=6,
                                                      name="wo_t")
                                    nc.sync.dma_start(
                                        wo_s[:],
                                        woT.ap()[1024 + 128 * k:
                                                 1024 + 128 * (k + 1),
                                                 512 * n:512 * (n + 1)])
                                    wo_t = wo_s[:]
                                for mp in range(4):
                                    nc.tensor.matmul(
                                        pso[(dn, mp)][:],
                                        av[:, k, 128 * mp:128 * (mp + 1)],
                                        wo_t,
                                        start=(h == 0 and k == 0),
                                        stop=(h == 1 and k == 7))
                    for dn in range(2):
                        n = 2 * ng + dn
                        for mp in range(4):
                            os_t = opool.tile([128, 512], F32, tag="osb",
                                              bufs=4, name="os_t")
                            nc.scalar.copy(os_t[:], pso[(dn, mp)][:])
                            nc.sync.dma_start(
                                out.ap()[128 * mp:128 * (mp + 1),
                                         512 * n:512 * (n + 1)], os_t[:])

    nc.compile()
    return nc


_NC_CACHE = None


def _get_nc():
    global _NC_CACHE
    if _NC_CACHE is None:
        _NC_CACHE = _build()
    return _NC_CACHE


def _host_prep(inputs):
    hs = np.asarray(inputs["hidden_states"], dtype=np.float32)
    Wq = np.asarray(inputs["Wq"], dtype=np.float32)
    Wk = np.asarray(inputs["Wk"], dtype=np.float32)
    Wv = np.asarray(inputs["Wv"], dtype=np.float32)
    Wo = np.asarray(inputs["Wo"], dtype=np.float32)
    cqw = np.asarray(inputs["canon_q_w"], dtype=np.float32)
    ckw = np.asarray(inputs["canon_k_w"], dtype=np.float32)
    cvw = np.asarray(inputs["canon_v_w"], dtype=np.float32)
    qnw = np.asarray(inputs["q_norm_w"], dtype=np.float32)
    knw = np.asarray(inputs["k_norm_w"], dtype=np.float32)

    bf = ml_dtypes.bfloat16
    hsT = np.ascontiguousarray(
        np.concatenate([hs[0].T, hs[1].T], axis=1)).astype(bf)
    WqT, WkT, WvT = Wq.T, Wk.T, Wv.T
    # Wo^T with even-head (h=0 per core) row-blocks first, then odd
    woT_full = Wo.T
    blocks = woT_full.reshape(16, 128, D)
    woT = np.ascontiguousarray(
        np.concatenate([blocks[0::2], blocks[1::2]], axis=0).reshape(D, D)
    ).astype(bf)

    inv_freq = 1.0 / (10000.0 ** (np.arange(0, DH, 2, dtype=np.float64) / DH))
    freqs = np.arange(S, dtype=np.float64)[:, None] * inv_freq
    emb = np.concatenate([freqs, freqs], axis=-1)
    cosT, sinT = np.cos(emb).T, np.sin(emb).T

    def make_rope(normw, scale):
        A = cosT * normw[:, None] * scale
        wswap = normw[(np.arange(DH) + 64) % DH]
        sign = np.where(np.arange(DH) < 64, -1.0, 1.0)
        Bc = sinT * wswap[:, None] * sign[:, None] * scale
        return (np.ascontiguousarray(A).astype(bf),
                np.ascontiguousarray(Bc).astype(bf))

    Aq, Bq = make_rope(qnw, SCALE)
    Ak, Bk = make_rope(knw, 1.0)

    p = np.arange(128)[:, None]
    f = np.arange(128)[None, :]
    maskd = np.where(p <= f, 0.0, NEG).astype(np.float32)

    in_maps = []
    for r in range(N_CORES):
        wTc = np.ascontiguousarray(np.concatenate(
            [WqT[:, 256 * r:256 * r + 256],
             WkT[:, 128 * r:128 * r + 128],
             WvT[:, 128 * r:128 * r + 128]], axis=1)).astype(bf)
        cwc = np.ascontiguousarray(np.concatenate(
            [cqw[256 * r:256 * r + 256],
             ckw[128 * r:128 * r + 128],
             cvw[128 * r:128 * r + 128]], axis=0)).astype(np.float32)
        in_maps.append({
            "hsT": hsT, "wT": wTc, "woT": woT, "cw": cwc,
            "ropeAq": Aq, "ropeBq": Bq, "ropeAk": Ak, "ropeBk": Bk,
            "maskd": maskd,
        })
    return in_maps


def kernel(**inputs):
    nc = _get_nc()
    in_maps = _host_prep(inputs)
    res = run_bass_kernel_spmd(nc, in_maps, core_ids=list(range(N_CORES)))
    full = np.empty((B, S, D), np.float32)
    for r in range(N_CORES):
        full[r // 4, 512 * (r % 4):512 * (r % 4 + 1), :] = res.results[r]["out"]
    return full


# revision 21
# speedup vs baseline: 1.4444x; 1.0783x over previous
"""Trainium2 Bass kernel for CanonCausalMultiheadAttn.

Sharding: tensor-parallel over heads across 8 cores (2 q-heads + 1 kv-head
per core), both batches replicated. Attention outputs are exchanged with two
head-split AllToAlls (h=0 fires while h=1 attention still computes); the
output projection contracts even-head dims first so it overlaps the second
collective.

Per-core pipeline (all shapes hardcoded for B=2, S=2048, D=2048):
  QKV proj (bf16 matmul, per 512-token chunk) -> canon conv at full row
  width (DVE, no chunk-boundary ops) -> qk rmsnorm via PE column-sum
  matmuls (squares on ACT; rstd via fast approx reciprocal; q's rstd
  broadcast with a K=1 outer-product matmul, k's rstd applied later as the
  per-partition `scale` of the exp activation) -> RoPE at full row width
  (DVE, norm-weight & 1/sqrt(dh) folded into host cos/sin tables) ->
  causal attention with scores in [Sk, Sq] layout; causal mask added by an
  identity-matmul accumulation on the PE; exp (ACT, bf16 out) ->
  transposed PV (stationary V block, moving P tile, output [d, Sq] in
  PSUM) with softmax denominators from ones-column matmuls; normalize via
  broadcast-reciprocal matmul + one DVE multiply per (pair, j) ->
  dual AllToAll -> output projection (bf16 matmul, even heads first).
"""
import sys

sys.path.insert(0, '/opt/trn_rl_repo')

import numpy as np
import ml_dtypes

import concourse.bass as bass
import concourse.mybir as mybir
import concourse.tile as tile
from concourse import bacc
from concourse.bass_utils import run_bass_kernel_spmd

F32 = mybir.dt.float32
F32R = mybir.dt.float32r
BF16 = mybir.dt.bfloat16
AF = mybir.ActivationFunctionType
ALU = mybir.AluOpType

B, S, D = 2, 2048, 2048
NH, NKV, DH = 16, 8, 128
K_CONV = 4
EPS = 1e-6
SCALE = 1.0 / float(np.sqrt(DH))
NEG = -1e9
N_CORES = 8
N_CHUNKS = S // 512
N_SKB = S // 128


def _build():
    nc = bacc.Bacc("TRN2", target_bir_lowering=False, debug=False,
                   num_devices=N_CORES)

    hsT = nc.dram_tensor("hsT", [D, B * S], BF16, kind="ExternalInput")
    wT = nc.dram_tensor("wT", [D, 512], BF16, kind="ExternalInput")
    woT = nc.dram_tensor("woT", [D, D], BF16, kind="ExternalInput")
    cw = nc.dram_tensor("cw", [512, K_CONV], F32, kind="ExternalInput")
    ropeAq = nc.dram_tensor("ropeAq", [DH, S], BF16, kind="ExternalInput")
    ropeBq = nc.dram_tensor("ropeBq", [DH, S], BF16, kind="ExternalInput")
    ropeAk = nc.dram_tensor("ropeAk", [DH, S], BF16, kind="ExternalInput")
    ropeBk = nc.dram_tensor("ropeBk", [DH, S], BF16, kind="ExternalInput")
    maskd = nc.dram_tensor("maskd", [128, 128], BF16, kind="ExternalInput")
    identd = nc.dram_tensor("identd", [128, 128], BF16, kind="ExternalInput")
    out = nc.dram_tensor("out", [512, D], F32, kind="ExternalOutput")

    with tile.TileContext(nc) as tc:
        with tc.tile_pool(name="const", bufs=1) as cpool, \
             tc.tile_pool(name="persist", bufs=1) as pers, \
             tc.tile_pool(name="dram", bufs=1, space="DRAM") as dram:

            # ---- constants (weight tile DMA leads the queue) ----
            wt_sb = pers.tile([128, 16 * 512], BF16, tag="wt", name="wt_sb")
            nc.sync.dma_start(
                wt_sb[:].rearrange("p (k s) -> p k s", s=512),
                wT.ap().rearrange("(k p) s -> p k s", p=128))
            wt = wt_sb[:].rearrange("p (k s) -> p k s", s=512)
            ropes = {}
            for nm in ("Aq", "Bq", "Ak", "Bk"):
                ropes[nm] = cpool.tile([DH, S], BF16, tag=f"rope{nm}",
                                       name=f"rope{nm}")
            mask_sb = cpool.tile([128, 128], BF16, tag="mask")
            nc.sync.dma_start(mask_sb[:], maskd.ap())
            iden_sb = cpool.tile([128, 128], BF16, tag="iden")
            nc.sync.dma_start(iden_sb[:], identd.ap())
            cw_sb = []
            for mt in range(4):
                t = cpool.tile([128, K_CONV], F32, tag=f"cw{mt}", name=f"cw{mt}")
                nc.sync.dma_start(t[:], cw.ap()[128 * mt:128 * mt + 128, :])
                cw_sb.append(t)
            ones_col_f = cpool.tile([128, 1], F32, tag="ocf")
            nc.vector.memset(ones_col_f[:], 1.0)
            ones_col_bf = cpool.tile([128, 1], BF16, tag="ocb")
            nc.scalar.copy(ones_col_bf[:], ones_col_f[:])
            ones_col = cpool.tile([128, 1], F32R, tag="oc")
            nc.scalar.copy(ones_col[:], ones_col_f[:])
            eps_sb = cpool.tile([1, 1], F32, tag="eps")
            nc.vector.memset(eps_sb[:], EPS)
            ones_row_f = cpool.tile([1, 128], F32, tag="orf")
            nc.vector.memset(ones_row_f[:], 1.0)
            ones_row = cpool.tile([1, 128], F32R, tag="or")
            nc.scalar.copy(ones_row[:], ones_row_f[:])
            s0_sb = []
            for mt in range(4):
                t = cpool.tile([128, 1], F32, tag=f"s0{mt}", name=f"s0{mt}")
                nc.vector.tensor_scalar_add(t[:], cw_sb[mt][:, 0:1], 1.0)
                s0_sb.append(t)

            # persistent per-(b,mt) outputs of the front-end
            roped = {}   # (b, mt<3) -> [128, S] bf16
            vaug = {}    # b -> [128, N_SKB*128] bf16, V in [Sk, d] blocks
            rstdkT = {}  # b -> [128, N_SKB] f32
            aout = {}    # h -> [128, 8*512] bf16, gathered attention
            for b in range(B):
                vaug[b] = pers.tile([128, N_SKB * 128], BF16, tag=f"vaug{b}",
                                    name=f"vaug{b}")
                rstdkT[b] = pers.tile([128, N_SKB], F32, tag=f"rstdkT{b}",
                                      name=f"rstdkT{b}")
                for mt in range(3):
                    roped[(b, mt)] = pers.tile([128, S], BF16,
                                               tag=f"roped{b}{mt}",
                                               name=f"roped{b}{mt}")
            for h in range(2):
                aout[h] = pers.tile([128, 8 * 512], BF16, tag=f"aout{h}",
                                    name=f"aout{h}")

            # ============ front-end: QKV + canon + norm + rope ============
            with tc.tile_pool(name="fwork", bufs=1) as fw, \
                 tc.tile_pool(name="qps", bufs=1, space="PSUM") as qps, \
                 tc.tile_pool(name="nps", bufs=2, space="PSUM") as nps, \
                 tc.tile_pool(name="bps", bufs=2, space="PSUM") as bps:
                rk_d = {}
                for b in range(B):
                    rk_d[b] = dram.tile([N_SKB, 128], F32, tag=f"rkd{b}",
                                        name=f"rk_d{b}")

                def qkv_chunk(b, n, raws):
                    lo = 512 * n
                    hs_sb = fw.tile([128, 16 * 512], BF16, tag="hschunk",
                                    bufs=2, name="hs_sb")
                    nc.sync.dma_start(
                        hs_sb[:].rearrange("p (k s) -> p k s", s=512),
                        hsT.ap()[:, b * S + lo:b * S + lo + 512]
                        .rearrange("(k p) s -> p k s", p=128))
                    if b == 0 and n == 0:
                        for nm, t in (("Aq", ropeAq), ("Bq", ropeBq),
                                      ("Ak", ropeAk), ("Bk", ropeBk)):
                            nc.sync.dma_start(ropes[nm][:], t.ap())
                    hv = hs_sb[:].rearrange("p (k s) -> p k s", s=512)
                    psums = [qps.tile([128, 512], F32, tag=f"qk{mt}",
                                      name=f"qk{mt}") for mt in range(4)]
                    for k in range(16):
                        for mt in range(4):
                            nc.tensor.matmul(
                                psums[mt][:],
                                wt[:, k, 128 * mt:128 * (mt + 1)],
                                hv[:, k, :],
                                start=(k == 0), stop=(k == 15))
                    for mt in range(4):
                        nc.scalar.copy(raws[mt][:, lo:lo + 512],
                                       psums[mt][:])

                def canon_full(b, raws, cfull):
                    # depthwise causal conv at full row width (residual
                    # folded into s0); no chunk-boundary special cases
                    for mt in range(4):
                        c = cfull[mt]
                        nc.vector.tensor_scalar_mul(c[:], raws[mt][:],
                                                    s0_sb[mt][:])
                        for k in range(1, K_CONV):
                            nc.vector.scalar_tensor_tensor(
                                c[:, k:S], raws[mt][:, 0:S - k],
                                cw_sb[mt][:, k:k + 1],
                                c[:, k:S], ALU.mult, ALU.add)
                    # V: transpose into [Sk, d] blocks
                    for i in range(N_SKB):
                        nc.sync.dma_start_transpose(
                            vaug[b][:, 128 * i:128 * (i + 1)],
                            cfull[3][:, 128 * i:128 * (i + 1)])

                def norm_mms(b, cfull, sq_sl, bcf):
                    # PE side of rmsnorm: sum-of-squares column matmuls and
                    # the q-rstd broadcast outer products
                    for mt in range(3):
                        is_q = mt < 2
                        for n in range(N_CHUNKS):
                            lo = 512 * n
                            sq = sq_sl[(mt, n)]
                            sp = nps.tile([1, 512], F32, tag="ssq")
                            nc.tensor.matmul(sp[:], ones_col[:], sq[:],
                                             start=True, stop=True)
                            srt = fw.tile([1, 512], F32, tag="srt", bufs=2,
                                          name="srt")
                            nc.scalar.activation(srt[:], sp[:], AF.Sqrt,
                                                 bias=eps_sb[:],
                                                 scale=1.0 / DH)
                            if is_q:
                                rq = fw.tile([1, 512], F32, tag="rq",
                                             bufs=2, name="rq")
                                nc.vector.reciprocal_approx_fast(rq[:],
                                                                 srt[:])
                                rqr = fw.tile([1, 512], F32R, tag="rqr",
                                              bufs=2, name="rqr")
                                nc.scalar.copy(rqr[:], rq[:])
                                bp = bps.tile([128, 512], F32, tag="bcp")
                                nc.tensor.matmul(bp[:], ones_row[:], rqr[:],
                                                 start=True, stop=True)
                                nc.scalar.copy(bcf[mt][:, lo:lo + 512],
                                               bp[:])
                            else:
                                rk = fw.tile([1, 512], F32, tag="rk",
                                             bufs=2, name="rk")
                                nc.vector.reciprocal_approx_fast(rk[:],
                                                                 srt[:])
                                nc.sync.dma_start(
                                    rk_d[b][4 * n:4 * (n + 1), :], rk[:])
                    nc.sync.dma_start(rstdkT[b][:],
                                      rk_d[b][:].rearrange("i p -> p i"))

                def squares(b, cfull, sq_sl):
                    # x*x on the ACT engine (keeps DVE free for canon/rope)
                    for mt in range(3):
                        for n in range(N_CHUNKS):
                            lo = 512 * n
                            sq = fw.tile([128, 512], F32R, tag="sqr",
                                         bufs=6, name="sq")
                            nc.scalar.activation(sq[:],
                                                 cfull[mt][:, lo:lo + 512],
                                                 AF.Square)
                            sq_sl[(mt, n)] = sq

                def rope_full(b, cfull, bcf):
                    for mt in range(3):
                        is_q = mt < 2
                        c = cfull[mt]
                        A_ = ropes["Aq"] if is_q else ropes["Ak"]
                        B_ = ropes["Bq"] if is_q else ropes["Bk"]
                        sh = fw.tile([128, S], BF16, tag="shift", bufs=1,
                                     name="sh")
                        nc.sync.dma_start(sh[0:64, :], c[64:128, :])
                        nc.sync.dma_start(sh[64:128, :], c[0:64, :])
                        nc.vector.tensor_mul(sh[:], sh[:], B_[:])
                        tm = fw.tile([128, S], BF16, tag="ropetmp", bufs=1,
                                     name="tm")
                        nc.vector.tensor_mul(tm[:], c[:], A_[:])
                        ro = roped[(b, mt)]
                        if is_q:
                            nc.vector.tensor_add(tm[:], tm[:], sh[:])
                            nc.vector.tensor_mul(ro[:], tm[:], bcf[mt][:])
                        else:
                            nc.vector.tensor_add(ro[:], tm[:], sh[:])

                # pipeline: b0 chunks | b1 c0 | b0 norm MMs | b1 c1-3 on the
                # PE while DVE runs b0 canon/rope one batch behind
                # shared tags with bufs=1: batch 1's tiles reuse batch 0's
                # buffers; the WAR deps line up with the natural pipeline
                # order (DVE runs b0's canon/rope before b1's canon)
                def mk_raws():
                    return {mt: fw.tile([128, S], BF16, tag=f"raw{mt}",
                                        name=f"raw{mt}") for mt in range(4)}

                def mk_cfull():
                    return {mt: fw.tile([128, S], BF16, tag=f"c{mt}",
                                        name=f"c{mt}") for mt in range(4)}

                def mk_bcf():
                    return {mt: fw.tile([128, S], BF16, tag=f"bc{mt}",
                                        name=f"bc{mt}") for mt in range(2)}

                sq_sl = {b: {} for b in range(B)}
                raws0 = mk_raws()
                for n in range(N_CHUNKS):
                    qkv_chunk(0, n, raws0)
                cfull0 = mk_cfull()
                canon_full(0, raws0, cfull0)
                squares(0, cfull0, sq_sl[0])
                raws1 = mk_raws()
                qkv_chunk(1, 0, raws1)
                bcf0 = mk_bcf()
                norm_mms(0, cfull0, sq_sl[0], bcf0)
                rope_full(0, cfull0, bcf0)
                for n in range(1, N_CHUNKS):
                    qkv_chunk(1, n, raws1)
                cfull1 = mk_cfull()
                canon_full(1, raws1, cfull1)
                squares(1, cfull1, sq_sl[1])
                bcf1 = mk_bcf()
                norm_mms(1, cfull1, sq_sl[1], bcf1)
                rope_full(1, cfull1, bcf1)

            # ======================= attention =======================
            a2a_in = [dram.tile([1024, 512], BF16, tag=f"a2ain{h}",
                                name=f"a2ain{h}") for h in range(2)]
            a2a_out = [dram.tile([1024, 512], BF16, tag=f"a2aout{h}",
                                 name=f"a2aout{h}") for h in range(2)]

            with tc.tile_pool(name="scps", bufs=2, space="PSUM") as scps, \
                 tc.tile_pool(name="atps", bufs=2, space="PSUM") as atps, \
                 tc.tile_pool(name="dsps", bufs=2, space="PSUM") as dsps, \
                 tc.tile_pool(name="bps2", bufs=2, space="PSUM") as bps2, \
                 tc.tile_pool(name="apool", bufs=1) as apool:
                for h in range(2):
                    for b in range(B):
                        KT = roped[(b, 2)]
                        va = vaug[b]
                        rkt = rstdkT[b]
                        QT = roped[(b, h)]
                        for j in range(N_CHUNKS):
                            ptiles = []
                            offs = []
                            for i in range(4 * j + 4):
                                r = i - 4 * j
                                off = 128 * max(r, 0)
                                ps = scps.tile([128, 512], F32, tag="sc",
                                               name="ps")
                                if r >= 0:
                                    # diagonal: mask folded in via an
                                    # identity-matmul accumulation
                                    nc.tensor.matmul(
                                        ps[:, off:off + 128],
                                        KT[:, 128 * i:128 * (i + 1)],
                                        QT[:, 512 * j + off:
                                           512 * j + off + 128],
                                        start=True, stop=False)
                                    nc.tensor.matmul(
                                        ps[:, off:off + 128],
                                        iden_sb[:], mask_sb[:],
                                        start=False, stop=True)
                                    if off + 128 < 512:
                                        nc.tensor.matmul(
                                            ps[:, off + 128:512],
                                            KT[:, 128 * i:128 * (i + 1)],
                                            QT[:, 512 * j + off + 128:
                                               512 * (j + 1)],
                                            start=True, stop=True)
                                else:
                                    nc.tensor.matmul(
                                        ps[:, :],
                                        KT[:, 128 * i:128 * (i + 1)],
                                        QT[:, 512 * j:512 * (j + 1)],
                                        start=True, stop=True)
                                pt = apool.tile([128, 512], BF16, tag="p",
                                                bufs=18, name="pt")
                                nc.scalar.activation(
                                    pt[:, off:512], ps[:, off:512], AF.Exp,
                                    scale=rkt[:, i:i + 1])
                                ptiles.append(pt)
                                offs.append(off)
                            ni = 4 * j + 4
                            # softmax denominators: ones-column matmuls
                            ds = dsps.tile([1, 512], F32, tag="ds",
                                           name="ds")
                            for i in range(ni):
                                nc.tensor.matmul(
                                    ds[:, offs[i]:512], ones_col_bf[:],
                                    ptiles[i][:, offs[i]:512],
                                    start=(i == 0), stop=(i == ni - 1))
                            # transposed PV: output [d, Sq] accumulated
                            at2 = atps.tile([128, 512], F32, tag="at",
                                            name="at2")
                            for i in range(ni):
                                nc.tensor.matmul(
                                    at2[:, offs[i]:512],
                                    va[:, 128 * i:128 * (i + 1)],
                                    ptiles[i][:, offs[i]:512],
                                    start=(i == 0), stop=(i == ni - 1))
                            rec = apool.tile([1, 512], F32, tag="rec",
                                             bufs=4, name="rec")
                            nc.vector.reciprocal_approx_fast(rec[:], ds[:])
                            recr = apool.tile([1, 512], F32R, tag="recr",
                                              bufs=4, name="recr")
                            nc.scalar.copy(recr[:], rec[:])
                            bcp = bps2.tile([128, 512], F32, tag="nb",
                                            name="bcp")
                            nc.tensor.matmul(bcp[:], ones_row[:], recr[:],
                                             start=True, stop=True)
                            bcs = apool.tile([128, 512], F32, tag="bcs",
                                             bufs=2, name="bcs")
                            nc.vector.tensor_scalar_mul(bcs[:], bcp[:], 1.0)
                            abf = apool.tile([128, 512], BF16, tag="abf",
                                             bufs=16, name="abf")
                            nc.vector.tensor_mul(abf[:], at2[:], bcs[:])
                            rd = 4 * b + j
                            nc.sync.dma_start(
                                a2a_in[h][128 * rd:128 * (rd + 1), :],
                                abf[:])

            # Collectives enqueued after all attention DMAs so their
            # Sync-queue completion fences can't strangle attention DMAs.
            # Trigger #0 only waits on a2a_in[0] writes -> overlaps h=1.
            for h in range(2):
                nc.gpsimd.collective_compute(
                    "AllToAll", ALU.bypass,
                    replica_groups=[list(range(N_CORES))],
                    ins=[a2a_in[h].opt()], outs=[a2a_out[h].opt()],
                    cc_dim="Partition")
                for k in range(8):
                    nc.sync.dma_start(
                        aout[h][:, 512 * k:512 * (k + 1)],
                        a2a_out[h][128 * k:128 * (k + 1), :])

            # ====================== out projection ====================
            # even-head (h=0) contraction first so it overlaps AllToAll #1;
            # Wo tiles stream on the gpsimd queue, immune to the Sync-queue
            # collective fences.
            with tc.tile_pool(name="opool", bufs=1) as opool, \
                 tc.tile_pool(name="ops", bufs=1, space="PSUM") as ops:
                for ng in range(2):     # n-pair groups: (0,1) then (2,3)
                    pso = {}
                    for dn in range(2):
                        for mp in range(4):
                            pso[(dn, mp)] = ops.tile(
                                [128, 512], F32, tag=f"o{dn}{mp}",
                                name=f"o{dn}{mp}")
                    for h in range(2):
                        av = aout[h][:].rearrange("p (k s) -> p k s", s=512)
                        for dn in range(2):
                            n = 2 * ng + dn
                            for k in range(8):
                                wo_s = opool.tile([128, 512], BF16,
                                                  tag="wo", bufs=6,
                                                  name="wo_t")
                                nc.gpsimd.dma_start(
                                    wo_s[:],
                                    woT.ap()[1024 * h + 128 * k:
                                             1024 * h + 128 * (k + 1),
                                             512 * n:512 * (n + 1)])
                                for mp in range(4):
                                    nc.tensor.matmul(
                                        pso[(dn, mp)][:],
                                        av[:, k, 128 * mp:128 * (mp + 1)],
                                        wo_s[:],
                                        start=(h == 0 and k == 0),
                                        stop=(h == 1 and k == 7))
                    for dn in range(2):
                        n = 2 * ng + dn
                        for mp in range(4):
                            os_t = opool.tile([128, 512], F32, tag="osb",
                                              bufs=4, name="os_t")
                            nc.scalar.copy(os_t[:], pso[(dn, mp)][:])
                            nc.sync.dma_start(
                                out.ap()[128 * mp:128 * (mp + 1),
                                         512 * n:512 * (n + 1)], os_t[:])

    nc.compile()
    return nc


_NC_CACHE = None


def _get_nc():
    global _NC_CACHE
    if _NC_CACHE is None:
        _NC_CACHE = _build()
    return _NC_CACHE


def _host_prep(inputs):
    hs = np.asarray(inputs["hidden_states"], dtype=np.float32)
    Wq = np.asarray(inputs["Wq"], dtype=np.float32)
    Wk = np.asarray(inputs["Wk"], dtype=np.float32)
    Wv = np.asarray(inputs["Wv"], dtype=np.float32)
    Wo = np.asarray(inputs["Wo"], dtype=np.float32)
    cqw = np.asarray(inputs["canon_q_w"], dtype=np.float32)
    ckw = np.asarray(inputs["canon_k_w"], dtype=np.float32)
    cvw = np.asarray(inputs["canon_v_w"], dtype=np.float32)
    qnw = np.asarray(inputs["q_norm_w"], dtype=np.float32)
    knw = np.asarray(inputs["k_norm_w"], dtype=np.float32)

    bf = ml_dtypes.bfloat16
    hsT = np.ascontiguousarray(
        np.concatenate([hs[0].T, hs[1].T], axis=1)).astype(bf)
    WqT, WkT, WvT = Wq.T, Wk.T, Wv.T
    # Wo^T with even-head (h=0 per core) row-blocks first, then odd
    woT_full = Wo.T
    blocks = woT_full.reshape(16, 128, D)
    woT = np.ascontiguousarray(
        np.concatenate([blocks[0::2], blocks[1::2]], axis=0).reshape(D, D)
    ).astype(bf)

    inv_freq = 1.0 / (10000.0 ** (np.arange(0, DH, 2, dtype=np.float64) / DH))
    freqs = np.arange(S, dtype=np.float64)[:, None] * inv_freq
    emb = np.concatenate([freqs, freqs], axis=-1)
    cosT, sinT = np.cos(emb).T, np.sin(emb).T

    def make_rope(normw, scale):
        A = cosT * normw[:, None] * scale
        wswap = normw[(np.arange(DH) + 64) % DH]
        sign = np.where(np.arange(DH) < 64, -1.0, 1.0)
        Bc = sinT * wswap[:, None] * sign[:, None] * scale
        return (np.ascontiguousarray(A).astype(bf),
                np.ascontiguousarray(Bc).astype(bf))

    Aq, Bq = make_rope(qnw, SCALE)
    Ak, Bk = make_rope(knw, 1.0)

    p = np.arange(128)[:, None]
    f = np.arange(128)[None, :]
    maskd = np.where(p <= f, 0.0, NEG).astype(bf)
    identd = np.eye(128, dtype=np.float32).astype(bf)

    in_maps = []
    for r in range(N_CORES):
        wTc = np.ascontiguousarray(np.concatenate(
            [WqT[:, 256 * r:256 * r + 256],
             WkT[:, 128 * r:128 * r + 128],
             WvT[:, 128 * r:128 * r + 128]], axis=1)).astype(bf)
        cwc = np.ascontiguousarray(np.concatenate(
            [cqw[256 * r:256 * r + 256],
             ckw[128 * r:128 * r + 128],
             cvw[128 * r:128 * r + 128]], axis=0)).astype(np.float32)
        in_maps.append({
            "hsT": hsT, "wT": wTc, "woT": woT, "cw": cwc,
            "ropeAq": Aq, "ropeBq": Bq, "ropeAk": Ak, "ropeBk": Bk,
            "maskd": maskd, "identd": identd,
        })
    return in_maps


def kernel(**inputs):
    nc = _get_nc()
    in_maps = _host_prep(inputs)
    res = run_bass_kernel_spmd(nc, in_maps, core_ids=list(range(N_CORES)))
    full = np.empty((B, S, D), np.float32)
    for r in range(N_CORES):
        full[r // 4, 512 * (r % 4):512 * (r % 4 + 1), :] = res.results[r]["out"]
    return full
